# revision 31
# baseline (speedup 1.0000x reference)
"""Trainium2 Bass kernel for ManualCausalSelfAttention.

Full (unsharded) inputs -> full output. Internally shards across 8
NeuronCores: data-parallel over batch (4) x tensor-parallel over head
groups of 8 heads (2). Each core computes a partial output projection
(its 512 rows of W_proj); the host sums the two partials per batch.

v3 design (vs v2 baseline, 318us):
  - PE warmup: ~48 tiny matmuls on a memset tile issued at t=0 warm the
    HAM clock gate (1.2->2.4GHz) while the weight/x DMAs land, so real
    matmuls start warm.
  - softmax denominator: gpsimd partition_broadcast + DVE reciprocal
    replace the DRAM transpose/broadcast bounce (5 serial sync-engine
    DMAs per pair -> 0).
  - single global software pipeline: attention runs with ST one key
    tile ahead of PV, and the projection of the NEXT token block plus
    the out-projection of the PREVIOUS query block are interleaved 2
    matmuls per key-tile-slot from a background queue, so the PE never
    idles while ScalarE runs the exp chain (scalar pace ~1010ns/kt vs
    852ns/kt of attention PE work).
  - psum->sbuf casts for q/k/v moved off ScalarE (to DVE) so scalar
    does exp only; psum budget: ST 2x[128,1024] + PV [65,1024] +
    2x[128,512] shared proj/outproj = 8 banks.
"""

import functools
import os
import sys
from collections import deque

import numpy as np

for _p in (
    "/root/.axon_site",
    "/root/.axon_site/_ro/trn_rl_repo",
    "/root/.axon_site/_ro/pypackages",
    "/opt/trn_rl_repo",
):
    if os.path.isdir(_p) and _p not in sys.path:
        sys.path.append(_p)

import concourse.bass as bass
import concourse.mybir as mybir
import concourse.tile as tile
from concourse.bass_utils import run_bass_kernel_spmd

# The PJRT boundary swallows python exceptions raised by the bass compile
# hook; wrap the hook so the real compile error is printed first.
from concourse import bass2jax as _b2j

if not getattr(_b2j, "_hook_wrapped", False):
    _orig_hook = _b2j.neuronx_cc_hook

    def _loud_hook(*a, **k):
        try:
            return _orig_hook(*a, **k)
        except BaseException:
            import traceback

            traceback.print_exc()
            raise

    _b2j.neuronx_cc_hook = _loud_hook
    _b2j._hook_wrapped = True

HID = 1024
NH_GLOBAL = 16
NHL = 8  # heads per core
D = 64
THETA = 10000.0
PAD_NEG = -60.0  # exp(scale*logit + PAD_NEG) == 0 relative to real keys
MASK_NEG = -480.0  # pre-scale causal bias: -480/8 == -60
F32 = mybir.dt.float32
BF16 = mybir.dt.bfloat16
ALU = mybir.AluOpType
AFT = mybir.ActivationFunctionType

TB = 512  # token block (phase 1, matmul N)
QB = 512  # query block (phase 2)

# stream_shuffle mask swapping adjacent partition pairs (per 32-group)
PAIR_SWAP_MASK = [i + 1 - 2 * (i % 2) for i in range(32)]


def _split_sync_waits(nc, max_waits=1):
    """walrus in this environment rejects instructions carrying more than
    `max_waits` sem waits; split the excess waits onto preceding
    same-engine NOPs."""
    for fn in nc.m.functions:
        for blk in fn.blocks:
            if not any(
                ins.sync_info is not None
                and ins.sync_info.on_wait
                and len(ins.sync_info.on_wait)
                > (0 if isinstance(ins, mybir.InstDrain) else max_waits)
                for ins in blk.instructions
            ):
                continue
            new_insts = []
            for ins in blk.instructions:
                si = ins.sync_info
                limit = 0 if isinstance(ins, mybir.InstDrain) else max_waits
                if si is not None and si.on_wait and len(si.on_wait) > limit:
                    waits = list(si.on_wait)
                    if limit:
                        extra, keep = waits[:-limit], waits[-limit:]
                    else:
                        extra, keep = waits, []
                    for i in range(0, len(extra), max_waits):
                        new_insts.append(
                            mybir.InstNoOp(
                                name=f"{ins.name}-sw{i}",
                                sync_info=mybir.SyncInfo(
                                    on_wait=extra[i : i + max_waits], on_update=[]
                                ),
                                bass_nofuse=True,
                                engine=ins.engine,
                            )
                        )
                    ins.sync_info = mybir.SyncInfo(
                        on_wait=keep, on_update=list(si.on_update)
                    )
                new_insts.append(ins)
            blk.instructions[:] = new_insts


def build_kernel(s=2048, split_waits=True, zero_bias=False):
    nc = bass.Bass()
    nb = s // TB      # token blocks
    nqb = s // QB     # query blocks
    nkt = s // 128    # 128-wide key tiles / token tiles
    hkt = HID // 128  # hidden contraction tiles
    assert nb == nqb

    xT = nc.dram_tensor("xt", [HID, s], BF16, kind="ExternalInput")
    w_qk = nc.dram_tensor("w_qk", [HID, 2 * NHL * D], BF16, kind="ExternalInput")
    w_v = nc.dram_tensor("w_v", [HID, NHL * D], BF16, kind="ExternalInput")
    w_pr = nc.dram_tensor("w_proj", [NHL * D, HID], BF16, kind="ExternalInput")
    cos_d = nc.dram_tensor("cos_t", [128, s], BF16, kind="ExternalInput")
    sin_d = nc.dram_tensor("sin_t", [128, s], BF16, kind="ExternalInput")
    bqk_d = nc.dram_tensor("b_qk", [128, 8], F32, kind="ExternalInput")
    bv_d = nc.dram_tensor("b_v", [1, NHL * D], BF16, kind="ExternalInput")
    bpr_d = nc.dram_tensor("b_proj", [1, HID], BF16, kind="ExternalInput")
    padm_d = nc.dram_tensor("pad_mul", [128, nkt], F32, kind="ExternalInput")
    padb_d = nc.dram_tensor("pad_bias", [128, nkt], F32, kind="ExternalInput")
    ones_d = nc.dram_tensor("ones_in", [128, 128], BF16, kind="ExternalInput")
    ident_d = nc.dram_tensor("ident", [128, 128], BF16, kind="ExternalInput")
    maskb_d = nc.dram_tensor("maskb", [128, 128], BF16, kind="ExternalInput")
    tri_d = nc.dram_tensor("trimask", [128, 256], BF16, kind="ExternalInput")
    z_d = nc.dram_tensor("z", [s, HID], BF16, kind="ExternalOutput")
    # DRAM bounce for softmax denominators: partitions can only be
    # crossed by DMA, so the den row goes out, comes back transposed onto
    # 128 partitions for a cheap exact reciprocal, goes out again, and
    # comes back broadcast across 64 partitions.
    den_scr = nc.dram_tensor("den_scr", [nqb * 4, 2 * QB], BF16)
    rcp_scr = nc.dram_tensor("rcp_scr", [nqb * 4, 2 * QB], F32)

    with tile.TileContext(nc) as tc:
        with (
            tc.tile_pool(name="persist", bufs=1) as ppool,
            tc.tile_pool(name="qkstore", bufs=1) as qkpool,
            tc.tile_pool(name="vstore", bufs=1) as vpool,
        ):
            padm_sb = ppool.tile([128, nkt], F32, tag="padm")
            padb_sb = ppool.tile([128, nkt], F32, tag="padb")
            ones_sb = ppool.tile([1, 128], BF16, tag="ones")
            ident_sb = ppool.tile([128, 128], BF16, tag="ident")
            maskb_sb = ppool.tile([128, 128], BF16, tag="maskb")
            tri_sb = ppool.tile([128, 2, 128], BF16, tag="tri")
            ones2_sb = ppool.tile([33, 64], BF16, tag="ones2")
            warm_sb = ppool.tile([128, 512], BF16, tag="warm")

            # q'T / k'T (RoPE'd, feature-major): 8 tiles of [128, s];
            # tiles 0..3 = Q head-pairs, 4..7 = K head-pairs.
            qk_store = [
                qkpool.tile([128, s], BF16, name=f"qk{mt}", tag=f"qk{mt}")
                for mt in range(8)
            ]
            # V with the denominator-ones column appended per head.
            v_sb = vpool.tile([128, nkt, NHL, D + 1], BF16, tag="v")

            with (
                tc.tile_pool(name="xt", bufs=2) as xpool,
                tc.tile_pool(name="c1", bufs=1) as c1pool,
                tc.tile_pool(name="ps_st", bufs=2, space="PSUM") as psST,
                tc.tile_pool(name="ps_pv", bufs=1, space="PSUM") as psPV,
                tc.tile_pool(name="ps_sm", bufs=2, space="PSUM") as psSM,
                tc.tile_pool(name="rope", bufs=3) as rpool,
                tc.tile_pool(name="pt", bufs=3) as ptpool,
                tc.tile_pool(name="ytn", bufs=12) as ytnpool,
                tc.tile_pool(name="yraw", bufs=2) as yrawpool,
                tc.tile_pool(name="small", bufs=2) as smpool,
                tc.tile_pool(name="zs", bufs=3) as zspool,
            ):
                # ---- PE warmup: memset a tile, then dummy matmuls keep
                # the PE busy (and the HAM clock warm) during the initial
                # weight/x DMA wait. The psST slot is reused by real ST
                # tiles afterwards.
                nc.vector.memset(warm_sb[:], 0.0)
                wps = psST.tile([128, 2 * QB], F32, tag="st")
                for _ in range(32):
                    nc.tensor.matmul(
                        wps[:, 0:128], warm_sb[:, 0:128], warm_sb[:, 0:128],
                        start=True, stop=True,
                    )
                for _ in range(12):
                    nc.tensor.matmul(
                        wps[:, 0:512], warm_sb[:, 0:128], warm_sb[:],
                        start=True, stop=True,
                    )

                w_qk_sb = c1pool.tile([128, hkt, 2 * NHL * D], BF16, tag="wqk")
                w_v_sb = c1pool.tile([128, hkt, NHL * D], BF16, tag="wv")
                cos_sb = c1pool.tile([128, s], BF16, tag="cos")
                sin_sb = c1pool.tile([128, s], BF16, tag="sin")
                bqk_sb = c1pool.tile([128, 8], F32, tag="bqk")
                bv_sb = c1pool.tile([1, NHL * D], BF16, tag="bv")
                w_pr_sb = c1pool.tile([128, NHL * D // 128, HID], BF16, tag="wpr")
                bpr_sb = c1pool.tile([1, HID], BF16, tag="bpr")
                xT_r = xT.rearrange("(kt p) t -> p kt t", p=128)
                w_qk_r = w_qk.rearrange("(kt p) f -> p kt f", p=128)
                # DMA priority order: first-needed first. The mt emission
                # order is (4,5,6,7,0,1,2,3) = column groups (2,3,0,1).
                for c in (2,):
                    nc.sync.dma_start(
                        w_qk_sb[:, :, c * 256 : (c + 1) * 256],
                        w_qk_r[:, :, c * 256 : (c + 1) * 256],
                    )
                xt0 = xpool.tile([128, hkt, TB], BF16, tag="xt")
                for i in range(4):
                    nc.sync.dma_start(
                        xt0[:, 2 * i : 2 * i + 2, :], xT_r[:, 2 * i : 2 * i + 2, 0:TB]
                    )
                for c in (3,):
                    nc.sync.dma_start(
                        w_qk_sb[:, :, c * 256 : (c + 1) * 256],
                        w_qk_r[:, :, c * 256 : (c + 1) * 256],
                    )
                nc.sync.dma_start(cos_sb[:], cos_d[:])
                nc.sync.dma_start(sin_sb[:], sin_d[:])
                for c in (0,):
                    nc.sync.dma_start(
                        w_qk_sb[:, :, c * 256 : (c + 1) * 256],
                        w_qk_r[:, :, c * 256 : (c + 1) * 256],
                    )
                nc.sync.dma_start(
                    w_v_sb[:, :, 0 : NHL * D // 2],
                    w_v.rearrange("(kt p) f -> p kt f", p=128)[:, :, 0 : NHL * D // 2],
                )
                for c in (1,):
                    nc.sync.dma_start(
                        w_qk_sb[:, :, c * 256 : (c + 1) * 256],
                        w_qk_r[:, :, c * 256 : (c + 1) * 256],
                    )
                nc.sync.dma_start(
                    w_v_sb[:, :, NHL * D // 2 :],
                    w_v.rearrange("(kt p) f -> p kt f", p=128)[:, :, NHL * D // 2 :],
                )
                nc.sync.dma_start(padm_sb[:], padm_d[:])
                nc.sync.dma_start(padb_sb[:], padb_d[:])
                nc.sync.dma_start(ones_sb[:], ones_d[0:1, 0:128])
                nc.sync.dma_start(ones2_sb[:], ones_d[0:33, 0:64])
                nc.sync.dma_start(ident_sb[:], ident_d[:])
                nc.sync.dma_start(maskb_sb[:], maskb_d[:])
                nc.sync.dma_start(
                    tri_sb[:], tri_d.rearrange("p (h q) -> p h q", h=2)
                )
                nc.sync.dma_start(bqk_sb[:], bqk_d[:])
                nc.sync.dma_start(bv_sb[:], bv_d[:])
                nc.sync.dma_start(
                    w_pr_sb[:], w_pr.rearrange("(sl p) f -> p sl f", p=128)
                )
                nc.sync.dma_start(bpr_sb[:], bpr_d[:])
                nc.vector.memset(v_sb[:, :, :, D : D + 1], 1.0)

                # ---------- background-work item generators ----------
                # Each item is (has_nonpe_tail, closure). A closure emits
                # one PE matmul; group-final closures also emit the
                # non-PE tail ops (casts / rope / store) for the group.

                def proj_items(nbi, xt):
                    """q/k/v projection of token block nbi: 12 groups of
                    hkt matmuls accumulating in a psSM tile."""
                    items = []
                    tsl = slice(nbi * TB, (nbi + 1) * TB)
                    qk_item_groups = {}
                    for mt in (4, 5, 6, 7, 0, 1, 2, 3):
                        cell = {}

                        def qk_mm(kt, mt=mt, cell=cell, xt=xt, tsl=tsl):
                            if kt == 0:
                                cell["ps"] = psSM.tile([128, TB], F32, name="psm", tag="sm")
                            ps = cell["ps"]
                            nc.tensor.matmul(
                                ps[:],
                                w_qk_sb[:, kt, mt * 128 : (mt + 1) * 128],
                                xt[:, kt, :],
                                start=(kt == 0),
                                stop=(kt == hkt - 1),
                            )
                            if kt == hkt - 1:
                                t1 = rpool.tile([128, TB], BF16, tag="t1")
                                t2p = rpool.tile([128, TB], BF16, tag="t2p")
                                t2 = rpool.tile([128, TB], BF16, tag="t2")
                                if zero_bias:
                                    # rope muls read psum directly (no
                                    # cast op); final add on gpsimd to
                                    # keep the DVE group cost below the
                                    # PE group cost
                                    nc.vector.tensor_mul(
                                        t1[:], ps[:], cos_sb[:, tsl]
                                    )
                                    nc.vector.tensor_mul(
                                        t2p[:], ps[:], sin_sb[:, tsl]
                                    )
                                    nc.vector.stream_shuffle(
                                        t2[:], t2p[:], PAIR_SWAP_MASK
                                    )
                                    nc.gpsimd.tensor_add(
                                        qk_store[mt][:, tsl], t1[:], t2[:]
                                    )
                                else:
                                    qraw = rpool.tile([128, TB], BF16, tag="qraw")
                                    nc.scalar.activation(
                                        qraw[:], ps[:], AFT.Identity,
                                        bias=bqk_sb[:, mt : mt + 1],
                                    )
                                    nc.vector.tensor_mul(
                                        t1[:], qraw[:], cos_sb[:, tsl]
                                    )
                                    nc.vector.tensor_mul(
                                        t2p[:], qraw[:], sin_sb[:, tsl]
                                    )
                                    nc.vector.stream_shuffle(
                                        t2[:], t2p[:], PAIR_SWAP_MASK
                                    )
                                    nc.vector.tensor_add(
                                        qk_store[mt][:, tsl], t1[:], t2[:]
                                    )

                        qk_item_groups[mt] = [
                            (kt == hkt - 1, functools.partial(qk_mm, kt))
                            for kt in range(hkt)
                        ]
                    for mt in (4, 5, 6, 7, 0, 1, 2, 3):
                        items.extend(qk_item_groups[mt])
                    for vt in range(TB // 128):
                        cell = {}

                        def v_mm(kt, vt=vt, cell=cell, xt=xt, nbi=nbi):
                            if kt == 0:
                                cell["ps"] = psSM.tile([128, TB], F32, name="psm", tag="sm")
                            psv = cell["ps"]
                            nc.tensor.matmul(
                                psv[:, 0 : NHL * D],
                                xt[:, kt, vt * 128 : (vt + 1) * 128],
                                w_v_sb[:, kt, :],
                                start=(kt == 0),
                                stop=(zero_bias and kt == hkt - 1),
                            )
                            if kt == hkt - 1:
                                if not zero_bias:
                                    nc.tensor.matmul(
                                        psv[:, 0 : NHL * D],
                                        ones_sb[:],
                                        bv_sb[:],
                                        start=False,
                                        stop=True,
                                    )
                                ktix = nbi * (TB // 128) + vt
                                nc.vector.tensor_copy(
                                    v_sb[:, ktix, :, 0:D],
                                    psv[:, 0 : NHL * D].rearrange(
                                        "p (h d) -> p h d", d=D
                                    ),
                                )

                        for kt in range(hkt):
                            items.append(
                                (kt == hkt - 1, functools.partial(v_mm, kt))
                            )
                    return items

                def outproj_items(qb, ytns, pair_order=(0, 1, 2, 3)):
                    """out-projection of query block qb: 8 groups (tt, ob)
                    of 4 matmuls (one per head-pair) + pad-scale + store."""
                    items = []
                    for tt in range(QB // 128):
                        for ob in range(HID // 512):
                            cell = {}

                            def o_mm(pi, tt=tt, ob=ob, cell=cell, qb=qb,
                                     ytns=ytns, pair_order=pair_order):
                                pair = pair_order[pi]
                                if pi == 0:
                                    cell["ps"] = psSM.tile([128, 512], F32, name="psm", tag="sm")
                                zp = cell["ps"]
                                last = pi == 3
                                nc.tensor.matmul(
                                    zp[:],
                                    ytns[pair][:, tt * 128 : (tt + 1) * 128],
                                    w_pr_sb[:, pair, ob * 512 : (ob + 1) * 512],
                                    start=(pi == 0),
                                    stop=(last and zero_bias),
                                )
                                if last:
                                    if not zero_bias:
                                        nc.tensor.matmul(
                                            zp[:],
                                            ones_sb[:],
                                            bpr_sb[:, ob * 512 : (ob + 1) * 512],
                                            start=False,
                                            stop=True,
                                        )
                                    gt = qb * (QB // 128) + tt
                                    zs = zspool.tile([128, 512], BF16, tag="zs")
                                    nc.vector.tensor_scalar(
                                        out=zs[:],
                                        in0=zp[:],
                                        scalar1=padm_sb[:, gt : gt + 1],
                                        scalar2=None,
                                        op0=ALU.mult,
                                    )
                                    nc.sync.dma_start(
                                        z_d[
                                            gt * 128 : (gt + 1) * 128,
                                            ob * 512 : (ob + 1) * 512,
                                        ],
                                        zs[:],
                                    )

                            for pi in range(4):
                                items.append(
                                    (pi == 3, functools.partial(o_mm, pi))
                                )
                    return items

                # ---------- attention building blocks ----------

                def emit_st(qb, pair, kt):
                    """score matmuls for one 128-wide key tile; returns
                    (psum tile, off) for the exp."""
                    dstart = qb * (QB // 128)
                    r = kt - dstart
                    off = max(r, 0) * 128
                    kst = qk_store[4 + pair]
                    qst = qk_store[pair]
                    stp = psST.tile([128, 2 * QB], F32, tag="st")
                    for h2 in (0, 1):
                        lo = h2 * 64
                        nc.tensor.matmul(
                            stp[:, h2 * QB + off : (h2 + 1) * QB],
                            kst[lo : lo + 64, kt * 128 : (kt + 1) * 128],
                            qst[lo : lo + 64, qb * QB + off : (qb + 1) * QB],
                            start=True,
                            stop=True,
                            tile_position=(lo, 0),
                        )
                    return stp, off

                def emit_exp(kt, stp, off, diag, mask_eng=None):
                    pt = ptpool.tile([128, 2, QB], BF16, tag="pt")
                    nc.scalar.activation(
                        pt[:, :, off:],
                        stp.rearrange("p (h q) -> p h q", h=2)[:, :, off:],
                        AFT.Exp,
                        bias=padb_sb[:, kt : kt + 1],
                        scale=float(D) ** -0.5,
                    )
                    if diag:
                        # zero the upper triangle of the diagonal 128-block
                        # (causal mask); engine picked per-block to balance
                        # DVE vs gpsimd load
                        (mask_eng or nc.vector).tensor_mul(
                            pt[:, :, off : off + 128],
                            pt[:, :, off : off + 128],
                            tri_sb[:],
                        )
                    return pt

                def emit_pv(pair, kt, ki, nkts, off, pt, cell):
                    if ki == 0:
                        cell["yp"] = psPV.tile(
                            [D + 1, 2 * QB], F32, name="pv", tag="pv"
                        )
                    yp = cell["yp"]
                    for h2 in (0, 1):
                        head = pair * 2 + h2
                        nc.tensor.matmul(
                            yp[:, h2 * QB + off : (h2 + 1) * QB],
                            v_sb[:, kt, head, :],
                            pt[:, h2, off:],
                            start=(ki == 0),
                            stop=(ki == nkts - 1),
                        )

                def emit_pair_end_tail(cell, ytns):
                    """last pair of the last block: transpose the den row
                    onto 128 partitions with tiny PE transposes, exact
                    reciprocal, transpose back, PE-broadcast across 64
                    partitions, and normalize on DVE reading the
                    broadcast directly from PSUM. No DMAs: ~4us chain
                    instead of ~12us, so the final out-projection starts
                    almost immediately."""
                    yp = cell["yp"]
                    denrow = smpool.tile([1, 2 * QB], BF16, name="denrow", tag="denrow")
                    nc.vector.tensor_copy(denrow[:], yp[D : D + 1, :])
                    dent_ps = psSM.tile([128, 1024], BF16, name="dentps", tag="sm")
                    for c in range(2 * QB // 128):
                        # 2-column spacing keeps each bf16 psum write
                        # 4-byte aligned
                        nc.tensor.transpose(
                            dent_ps[:, 2 * c : 2 * c + 1],
                            denrow[0:1, c * 128 : (c + 1) * 128],
                            ones2_sb[0:1, 0:1],
                        )
                    rcpt = smpool.tile([128, 2 * QB // 128], BF16, name="rcpt2", tag="rcpt2")
                    dent_v = dent_ps[:, 0 : 4 * QB // 128].rearrange(
                        "p (c two) -> p c two", two=2
                    )[:, :, 0:1]
                    with nc.allow_low_precision(reason="bf16 recip matches the den path precision"):
                        nc.vector.reciprocal(rcpt[:], dent_v)
                    rcpr_ps = psSM.tile([128, 1024], BF16, name="rcprps", tag="sm")
                    for h2 in (0, 1):
                        for c in range(QB // 128):
                            nc.tensor.transpose(
                                rcpr_ps[h2 * 32 : h2 * 32 + 1,
                                        c * 128 : (c + 1) * 128],
                                rcpt[:, h2 * 4 + c : h2 * 4 + c + 1],
                                ident_sb[:],
                            )
                    rcpr_sb = smpool.tile([33, QB], BF16, name="rcprsb", tag="rcprsb")
                    for h2 in (0, 1):
                        nc.vector.tensor_copy(
                            rcpr_sb[h2 * 32 : h2 * 32 + 1, :],
                            rcpr_ps[h2 * 32 : h2 * 32 + 1, 0:QB],
                        )
                    yraw = yrawpool.tile([D + 1, 2 * QB], BF16, tag="yraw")
                    nc.vector.tensor_copy(yraw[:], yp[:])
                    ytn = ytnpool.tile([128, QB], BF16, tag="ytn")
                    for h2 in (0, 1):
                        denb = psSM.tile([64, 512], F32, name="denb", tag="sm")
                        nc.tensor.matmul(
                            denb[:],
                            ones2_sb[h2 * 32 : h2 * 32 + 1, :],
                            rcpr_sb[h2 * 32 : h2 * 32 + 1, :],
                            start=True,
                            stop=True,
                        )
                        nc.vector.tensor_mul(
                            ytn[h2 * 64 : (h2 + 1) * 64, :],
                            yraw[0:D, h2 * QB : (h2 + 1) * QB],
                            denb[:],
                        )
                    ytns.append(ytn)

                def emit_pair_end(qb, pair, cell, ytns, on_dve=False):
                    """softmax denominator + normalize for one head pair:
                    evacuate psum, bounce the den row through DRAM to
                    transpose it onto 128 partitions (cheap exact DVE
                    reciprocal), bounce back broadcast, normalize on
                    gpsimd. Hops 1-2 issue from the gpsimd queue, hops
                    3-4 from sync, so no single queue serializes the
                    four pairs' chains."""
                    yp = cell["yp"]
                    row = qb * 4 + pair
                    yraw = yrawpool.tile([D + 1, 2 * QB], BF16, tag="yraw")
                    nc.vector.tensor_copy(yraw[:], yp[:])
                    nc.gpsimd.dma_start(
                        den_scr[row : row + 1, :], yraw[D : D + 1, :]
                    )
                    dent = smpool.tile([128, 2 * QB // 128], BF16, name="dent", tag="dent")
                    nc.gpsimd.dma_start(
                        dent[:],
                        den_scr[row : row + 1, :].rearrange(
                            "o (p f) -> (o p) f", p=128
                        ),
                    )
                    rcpt = smpool.tile([128, 2 * QB // 128], F32, name="rcpt", tag="rcpt")
                    nc.vector.reciprocal(rcpt[:], dent[:])
                    nc.sync.dma_start(
                        rcp_scr[row : row + 1, :].rearrange(
                            "o (p f) -> (o p) f", p=128
                        ),
                        rcpt[:],
                    )
                    rb = smpool.tile([64, 2 * QB], F32, name="rb", tag="rb")
                    nc.sync.dma_start(
                        rb[:], rcp_scr[row : row + 1, :].broadcast_to([64, 2 * QB])
                    )
                    ytn = ytnpool.tile([128, QB], BF16, tag="ytn")
                    eng = nc.vector if on_dve else nc.gpsimd
                    for h2 in (0, 1):
                        eng.tensor_mul(
                            ytn[h2 * 64 : (h2 + 1) * 64, :],
                            yraw[0:D, h2 * QB : (h2 + 1) * QB],
                            rb[:, h2 * QB : (h2 + 1) * QB],
                        )
                    ytns.append(ytn)

                # ---------- the fused main loop ----------
                bg = deque()

                def drain_bg(n, allow_tail=True):
                    while n > 0 and bg:
                        has_tail, fn = bg[0]
                        if has_tail and not allow_tail:
                            break
                        bg.popleft()
                        fn()
                        n -= 1

                prev_ytns = None
                pprev_ytns = None
                xt_cur = xt0
                xt_next = None
                for qb in range(nqb):
                    if qb == 0:
                        # projection of block 0 runs serial (nothing to
                        # overlap with yet)
                        for _, fn in proj_items(0, xt_cur):
                            fn()
                    if qb + 1 < nb:
                        # prefetch x for block qb+1 and queue its
                        # projection into the attention of qb
                        xt_next = xpool.tile([128, hkt, TB], BF16, tag="xt")
                        nc.sync.dma_start(
                            xt_next[:],
                            xT_r[:, :, (qb + 1) * TB : (qb + 2) * TB],
                        )
                        bg.extend(proj_items(qb + 1, xt_next))
                    if prev_ytns is not None:
                        if qb == nqb - 1:
                            # the last query block has no projection work
                            # to interleave; both pending out-projections
                            # fill its 64 slots (1 per slot) so the PE
                            # stays dense (and the clock warm) to the end
                            bg.extend(outproj_items(qb - 2, pprev_ytns))
                            bg.extend(outproj_items(qb - 1, prev_ytns))
                        elif qb >= 2:
                            pass  # deferred to the last block
                        else:
                            bg.extend(outproj_items(qb - 1, prev_ytns))

                    nkts = (qb + 1) * (QB // 128)
                    pair_order = (3, 0, 1, 2) if qb == nqb - 1 else (0, 1, 2, 3)
                    seq = [(pair, kt) for pair in pair_order for kt in range(nkts)]
                    cells = {pair: {} for pair in pair_order}
                    # prologue: ST+exp of the first slot
                    dstart = qb * (QB // 128)
                    stp0, off0 = emit_st(qb, seq[0][0], seq[0][1])
                    pts = {}
                    mask_eng = nc.vector
                    pts[0] = emit_exp(seq[0][1], stp0, off0,
                                      seq[0][1] >= dstart, mask_eng)
                    offs = {0: off0}
                    for i, (pair, kt) in enumerate(seq):
                        if i + 1 < len(seq):
                            npair, nkt = seq[i + 1]
                            stp, off = emit_st(qb, npair, nkt)
                            pts[i + 1] = emit_exp(nkt, stp, off,
                                                  nkt >= dstart, mask_eng)
                            offs[i + 1] = off
                        # 2 background matmuls per slot fill the PE while
                        # scalar works; suppress group-tail items near the
                        # pair end so the DVE copy isn't queued behind them
                        near_end = kt >= nkts - 2
                        slots_left = len(seq) - i
                        dens = min(6, max(1, -(-len(bg) // slots_left)))
                        drain_bg(dens, allow_tail=not near_end)
                        ki = kt
                        emit_pv(pair, kt, ki, nkts, offs[i], pts.pop(i), cells[pair])
                        if kt == nkts - 1:
                            if qb == nqb - 1 and pair == pair_order[-1]:
                                emit_pair_end_tail(cells[pair], ytns := [])
                            else:
                                emit_pair_end(qb, pair, cells[pair], ytns := [])
                            if pair == pair_order[0]:
                                all_ytns = [None] * 4
                            all_ytns[pair] = ytns[0]
                    # drain whatever background work remains before the
                    # next query block
                    drain_bg(len(bg))
                    pprev_ytns = prev_ytns
                    prev_ytns = all_ytns
                    xt_cur = xt_next

                # ---- final out-projection (query block nqb-1) ----
                # The last pair's softmax-denominator chain (pair 2 with
                # the (3,0,1,2) pair order) takes ~14us of cross-engine
                # latency after the last PV. Use the now-free psST pool
                # ([128, 2QB] tiles, 2 bufs) for per-tt groups whose
                # "ready" matmuls (pairs 3,0,1 of both ob halves) issue
                # immediately, and keep the clock warm with dummy warm
                # matmuls while waiting for pair 2's normalization.
                fyt = all_ytns

                def fgroup_ready(tt, cell):
                    cell["zp"] = psST.tile([128, 2 * QB], F32, name="fzp", tag="st")
                    zp = cell["zp"]
                    for ob in range(2):
                        for pair in (3, 0, 1):
                            nc.tensor.matmul(
                                zp[:, ob * 512 : (ob + 1) * 512],
                                fyt[pair][:, tt * 128 : (tt + 1) * 128],
                                w_pr_sb[:, pair, ob * 512 : (ob + 1) * 512],
                                start=(pair == 3),
                                stop=False,
                            )

                def fgroup_close(tt, cell):
                    zp = cell["zp"]
                    gt = (nqb - 1) * (QB // 128) + tt
                    for ob in range(2):
                        nc.tensor.matmul(
                            zp[:, ob * 512 : (ob + 1) * 512],
                            fyt[2][:, tt * 128 : (tt + 1) * 128],
                            w_pr_sb[:, 2, ob * 512 : (ob + 1) * 512],
                            start=False,
                            stop=zero_bias,
                        )
                        if not zero_bias:
                            nc.tensor.matmul(
                                zp[:, ob * 512 : (ob + 1) * 512],
                                ones_sb[:],
                                bpr_sb[:, ob * 512 : (ob + 1) * 512],
                                start=False,
                                stop=True,
                            )
                    for ob in range(2):
                        zs = zspool.tile([128, 512], BF16, tag="zs")
                        nc.vector.tensor_scalar(
                            out=zs[:], in0=zp[:, ob * 512 : (ob + 1) * 512],
                            scalar1=padm_sb[:, gt : gt + 1],
                            scalar2=None, op0=ALU.mult,
                        )
                        nc.sync.dma_start(
                            z_d[gt * 128 : (gt + 1) * 128,
                                ob * 512 : (ob + 1) * 512],
                            zs[:],
                        )

                # sub-groups for tt 2,3 on the psSM pool ([128,512]
                # per (tt,ob)) so their ready matmuls also issue under
                # the chain; warm filler runs in the freed psPV bank.
                def sub_ready(tt, ob, cell):
                    cell["zp"] = psSM.tile([128, 512], F32, name="fsz", tag="sm")
                    zp = cell["zp"]
                    for pair in (3, 0, 1):
                        nc.tensor.matmul(
                            zp[:],
                            fyt[pair][:, tt * 128 : (tt + 1) * 128],
                            w_pr_sb[:, pair, ob * 512 : (ob + 1) * 512],
                            start=(pair == 3),
                            stop=False,
                        )

                def sub_close(tt, ob, cell):
                    zp = cell["zp"]
                    gt = (nqb - 1) * (QB // 128) + tt
                    nc.tensor.matmul(
                        zp[:],
                        fyt[2][:, tt * 128 : (tt + 1) * 128],
                        w_pr_sb[:, 2, ob * 512 : (ob + 1) * 512],
                        start=False,
                        stop=zero_bias,
                    )
                    if not zero_bias:
                        nc.tensor.matmul(
                            zp[:], ones_sb[:],
                            bpr_sb[:, ob * 512 : (ob + 1) * 512],
                            start=False, stop=True,
                        )
                    zs = zspool.tile([128, 512], BF16, tag="zs")
                    nc.vector.tensor_scalar(
                        out=zs[:], in0=zp[:],
                        scalar1=padm_sb[:, gt : gt + 1],
                        scalar2=None, op0=ALU.mult,
                    )
                    nc.sync.dma_start(
                        z_d[gt * 128 : (gt + 1) * 128,
                            ob * 512 : (ob + 1) * 512],
                        zs[:],
                    )

                warmps = psPV.tile([128, 512], F32, name="warmps", tag="pv")

                def warm(n):
                    for _ in range(n):
                        nc.tensor.matmul(
                            warmps[:], warm_sb[:, 0:128], warm_sb[:],
                            start=True, stop=True,
                        )

                fcells = [{} for _ in range(QB // 128)]
                scells = {(tt, ob): {} for tt in (2, 3) for ob in (0, 1)}
                fgroup_ready(0, fcells[0])
                fgroup_ready(1, fcells[1])
                sub_ready(2, 0, scells[(2, 0)])
                sub_ready(2, 1, scells[(2, 1)])
                warm(18)
                fgroup_close(0, fcells[0])
                fgroup_close(1, fcells[1])
                sub_close(2, 0, scells[(2, 0)])
                sub_close(2, 1, scells[(2, 1)])
                sub_ready(3, 0, scells[(3, 0)])
                sub_close(3, 0, scells[(3, 0)])
                sub_ready(3, 1, scells[(3, 1)])
                sub_close(3, 1, scells[(3, 1)])
    if split_waits:
        _split_sync_waits(nc)
    return nc


@functools.lru_cache(maxsize=2)
def _built(s, zero_bias=False):
    return build_kernel(s, zero_bias=zero_bias)


def _rope_tables(s):
    j = np.arange(D // 2, dtype=np.float64)
    inv = THETA ** (-2.0 * j / D)
    ang = np.arange(s, dtype=np.float64)[:, None] * inv[None, :]  # [s, 32]
    cos = np.cos(ang).T  # [32, s]
    sin = np.sin(ang).T
    cos64 = np.repeat(cos, 2, axis=0)  # rows 2j, 2j+1 identical
    sin64 = np.repeat(sin, 2, axis=0)
    # "pre-swap" sign convention: the kernel multiplies by this table BEFORE
    # pair-swapping partitions, so odd rows carry the minus sign.
    sin64[1::2, :] *= -1.0
    cos128 = np.concatenate([cos64, cos64], axis=0)
    sin128 = np.concatenate([sin64, sin64], axis=0)
    return np.ascontiguousarray(cos128), np.ascontiguousarray(sin128)


def _col_tiled(vec, tile_rows=128):
    """[n] -> [tile_rows, n//tile_rows], column t = vec[t*128:(t+1)*128]."""
    n = vec.shape[0]
    return np.ascontiguousarray(vec.reshape(n // tile_rows, tile_rows).T)


def _bf16(a):
    import ml_dtypes

    return np.ascontiguousarray(np.asarray(a).astype(ml_dtypes.bfloat16))


def make_in_maps(x, attention_padding, W_qkv, b_qkv, W_proj, b_proj):
    x = np.asarray(x, dtype=np.float32)
    pad = np.asarray(attention_padding).astype(bool)
    W_qkv = np.asarray(W_qkv, dtype=np.float32)
    b_qkv = np.asarray(b_qkv, dtype=np.float32)
    W_proj = np.asarray(W_proj, dtype=np.float32)
    b_proj = np.asarray(b_proj, dtype=np.float32)
    B, s, _ = x.shape
    cos128, sin128 = _rope_tables(s)
    cos128_bf = _bf16(cos128)
    sin128_bf = _bf16(sin128)
    ident = np.eye(128, dtype=np.float32)
    ones = np.ones((128, 128), dtype=np.float32)
    maskb = np.where(
        np.arange(128)[None, :] < np.arange(128)[:, None], MASK_NEG, 0.0
    ).astype(np.float32)
    tri = np.where(
        np.arange(128)[None, :] >= np.arange(128)[:, None], 1.0, 0.0
    ).astype(np.float32)
    tri2 = np.concatenate([tri, tri], axis=1)

    per_hp = {}
    for hp in range(2):
        hs = slice(hp * NHL * D, (hp + 1) * NHL * D)
        Wq = W_qkv[:, 0:HID][:, hs]
        Wk = W_qkv[:, HID : 2 * HID][:, hs]
        Wv = W_qkv[:, 2 * HID : 3 * HID][:, hs]
        bq = b_qkv[0:HID][hs]
        bk = b_qkv[HID : 2 * HID][hs]
        bv = b_qkv[2 * HID : 3 * HID][hs]
        bqk = np.concatenate([bq, bk])
        per_hp[hp] = dict(
            w_qk=_bf16(np.concatenate([Wq, Wk], axis=1)),
            w_v=_bf16(Wv),
            w_proj=_bf16(W_proj[hs, :]),
            b_qk=_col_tiled(bqk),
            b_v=_bf16(bv[None, :]),
            b_proj=_bf16(
                (b_proj if hp == 0 else np.zeros_like(b_proj))[None, :]
            ),
        )

    per_b = {}
    for b in range(B):
        p = pad[b].astype(np.float32)
        per_b[b] = dict(
            xt=_bf16(x[b].T),
            pad_mul=_col_tiled(p),
            pad_bias=_col_tiled(np.where(pad[b], 0.0, PAD_NEG).astype(np.float32)),
        )

    in_maps = []
    for c in range(2 * B):
        b, hp = c // 2, c % 2
        m = dict(per_hp[hp])
        m.update(per_b[b])
        m["cos_t"] = cos128_bf
        m["sin_t"] = sin128_bf
        m["ones_in"] = _bf16(ones)
        m["ident"] = _bf16(ident)
        m["maskb"] = _bf16(maskb)
        m["trimask"] = _bf16(tri2)
        in_maps.append(m)
    return in_maps


def run(x, attention_padding, W_qkv, b_qkv, W_proj, b_proj, trace=False, **spmd_kw):
    x = np.asarray(x, dtype=np.float32)
    B, s, _ = x.shape
    zero_bias = bool(
        np.all(np.asarray(b_qkv) == 0) and np.all(np.asarray(b_proj) == 0)
    )
    nc = _built(s, zero_bias)
    in_maps = make_in_maps(x, attention_padding, W_qkv, b_qkv, W_proj, b_proj)
    res = run_bass_kernel_spmd(nc, in_maps, list(range(2 * B)), trace=trace, **spmd_kw)
    out = np.stack(
        [
            np.asarray(res.results[2 * b]["z"], dtype=np.float32)
            + np.asarray(res.results[2 * b + 1]["z"], dtype=np.float32)
            for b in range(B)
        ]
    )
    return out, res


def kernel(x, attention_padding, W_qkv, b_qkv, W_proj, b_proj, train=None, **_):
    out, _res = run(x, attention_padding, W_qkv, b_qkv, W_proj, b_proj)
    return out
